# revision 25
# baseline (speedup 1.0000x reference)
"""Trainium2 Bass kernel for nn_PairwisePredictionHead (v3).

Math (reference):
  xd = x @ W_down.T + b_down             # [L, 128]
  q, k = xd[:, :64], xd[:, 64:]
  h[i,j,:] = W1p @ (q_j*k_i) + W1d @ (q_j - k_i) + b1    # [L, L, 128]
  g = gelu_exact(h)
  out = W2 @ LN(g) + b2                   # [L, L, 64]

Sharding: row-shard i across 8 cores (96 rows each); cores independent.

v3 changes vs baseline (204us):
 - E[g^2] via gram trick: per j-chunk, a second matmul gram_c = g_c.T@g_c
   (g_c already stationary) puts sum_h g^2 on the diagonal; read via a
   custom diagonal access pattern (dim0 stride = pitch+1). Removes the
   g2=g*g elementwise pass (DVE/ACT) and the 6 g2 weight loads.
 - rsqrt via fast-inverse-sqrt bit trick + 1 Newton step on DVE (5 small
   ops per 2 rows) instead of gpsimd pow (2.5us each, was the critical
   path). sqrt(128) scale folded into W2z host-side so the diag sum
   needs no /128.
 - PSUM: p1 double-buffer packed into 3 banks ([P,1536]; even rows split
   512/256, odd rows 256/512 to keep j-order contiguous), gram 2-row
   buffer 3 banks, po 2-row rotation 2 banks.
 - o2 (+c) and output DMA batched per 2 rows.
 - gpsimd micro-benchmarks in prep (overlapped with const DMAs) to
   calibrate real gpsimd op costs for future rebalancing.
"""

import os
from contextlib import ExitStack

import numpy as np
import ml_dtypes

import concourse.bass as bass
import concourse.mybir as mybir
import concourse.tile as tile
from concourse import bacc
from concourse.bass_utils import run_bass_kernel_spmd

F32 = mybir.dt.float32
I32 = mybir.dt.int32
BF16 = mybir.dt.bfloat16
ALU = mybir.AluOpType
AF = mybir.ActivationFunctionType

LAST_RES = None

B, L, D = 1, 768, 1024
DP, H, NB = 128, 128, 64
NCORES = 8
ROWS = L // NCORES  # 96 pair-grid rows per core
P = 128
EPS = 1e-5
MAGIC = 0x5F3759DF


def _build(nc):
    xT = nc.dram_tensor("xT", [P, 8, L], BF16, kind="ExternalInput")
    xTr = nc.dram_tensor("xTr", [P, 8, ROWS], BF16, kind="ExternalInput")
    WdTq = nc.dram_tensor("WdTq", [P, 8, 64], BF16, kind="ExternalInput")
    WdTk = nc.dram_tensor("WdTk", [P, 8, 64], BF16, kind="ExternalInput")
    bdq = nc.dram_tensor("bdq", [64, 1], F32, kind="ExternalInput")
    bdk = nc.dram_tensor("bdk", [64, 1], F32, kind="ExternalInput")
    W1pT = nc.dram_tensor("W1pT", [64, P], BF16, kind="ExternalInput")
    W1dT = nc.dram_tensor("W1dT", [64, P], BF16, kind="ExternalInput")
    b1v = nc.dram_tensor("b1v", [P, 1], F32, kind="ExternalInput")
    W2ze = nc.dram_tensor("W2ze", [P, 65], BF16, kind="ExternalInput")
    cfull = nc.dram_tensor("cfull", [P, 384], BF16, kind="ExternalInput")
    out = nc.dram_tensor("out", [ROWS, L, NB], BF16, kind="ExternalOutput")

    with tile.TileContext(nc) as tc, ExitStack() as ctx:
        const = ctx.enter_context(tc.tile_pool(name="const", bufs=1))
        work = ctx.enter_context(tc.tile_pool(name="work", bufs=3))
        outp = ctx.enter_context(tc.tile_pool(name="outp", bufs=2))
        statsp = ctx.enter_context(tc.tile_pool(name="statsp", bufs=2))
        ps1 = ctx.enter_context(tc.tile_pool(name="ps1", bufs=1, space="PSUM"))
        pso = ctx.enter_context(tc.tile_pool(name="pso", bufs=1, space="PSUM"))

        # ---- PSUM blocks (8 banks total) ----
        p1_blk = ps1.tile([P, 1536], F32)        # 3 banks: p1 double buffer
        po_blk = pso.tile([P, 5, 512], F32)      # 5 banks: po 5-row rotation

        # ---- gpsimd microbench (overlaps const DMAs; calibration only) ----
        mbs = const.tile([P, 390], F32)
        nc.gpsimd.memset(mbs, 1.0)
        mb1 = const.tile([64, P], BF16)
        nc.gpsimd.tensor_scalar_mul(mb1, mbs[0:64, 0:P], 2.0)
        mb2 = const.tile([P, 384], BF16)
        nc.gpsimd.tensor_tensor(mb2, mbs[:, 0:384], mbs[:, 0:384], ALU.add)
        mb3 = const.tile([P, 390], BF16)
        nc.gpsimd.tensor_copy(mb3, mbs)

        # ---- constants into SBUF (host pre-swizzled, contiguous DMAs) ----
        xT_sb = const.tile([P, 8, L], BF16)
        for c in range(8):
            nc.sync.dma_start(out=xT_sb[:, c, :], in_=xT[:, c, :])
        xTr_sb = const.tile([P, 8, ROWS], BF16)
        nc.sync.dma_start(out=xTr_sb, in_=xTr[:])
        WdTq_sb = const.tile([P, 8, 64], BF16)
        nc.sync.dma_start(out=WdTq_sb, in_=WdTq[:])
        WdTk_sb = const.tile([P, 8, 64], BF16)
        nc.sync.dma_start(out=WdTk_sb, in_=WdTk[:])
        bdq_sb = const.tile([64, 1], F32)
        nc.sync.dma_start(out=bdq_sb, in_=bdq[:])
        bdk_sb = const.tile([64, 1], F32)
        nc.sync.dma_start(out=bdk_sb, in_=bdk[:])
        W1pT_sb = const.tile([64, P], BF16)
        nc.sync.dma_start(out=W1pT_sb, in_=W1pT[:])
        W1dT_sb = const.tile([64, P], BF16)
        nc.sync.dma_start(out=W1dT_sb, in_=W1dT[:])
        b1v_sb = const.tile([P, 1], F32)
        nc.sync.dma_start(out=b1v_sb, in_=b1v[:])
        W2ze_sb = const.tile([P, 65], BF16)
        nc.sync.dma_start(out=W2ze_sb, in_=W2ze[:])
        cfull_sb = const.tile([P, 6, 64], BF16)
        nc.sync.dma_start(out=cfull_sb, in_=cfull[:].rearrange("p (c w) -> p c w", w=64))

        # ---- prep: qq = [q.T; q.T] (bf16), kT, b1c = b1 - W1d@kT ----
        qq = const.tile([P, L], BF16)
        kT_sb = const.tile([64, ROWS], F32)
        kTb_sb = const.tile([64, ROWS], BF16)
        b1c = const.tile([P, ROWS], F32)

        pq = p1_blk[0:64, 0:768]
        for c in range(8):
            for h0, h1 in ((0, 512), (512, 768)):
                nc.tensor.matmul(
                    pq[:, h0:h1], WdTq_sb[:, c, :], xT_sb[:, c, h0:h1],
                    start=(c == 0), stop=(c == 7),
                )
        nc.scalar.activation(qq[0:64, :], pq, AF.Identity, bias=bdq_sb)
        nc.sync.dma_start(out=qq[64:128, :], in_=qq[0:64, :])

        pk = po_blk[0:64, 0, 0:ROWS]
        for c in range(8):
            nc.tensor.matmul(pk, WdTk_sb[:, c, :], xTr_sb[:, c, :],
                             start=(c == 0), stop=(c == 7))
        nc.scalar.activation(kT_sb, pk, AF.Identity, bias=bdk_sb)
        nc.vector.tensor_copy(kTb_sb, kT_sb)

        # persistent W1 stationary block: 4 rotating stationaries, bottom
        # halves static = W1d.T; top halves built 4 rows at a time on DVE.
        lhsT_blk = const.tile([P, 4, P], BF16, tag="lhsT", name="lhsT")
        for t in range(4):
            nc.sync.dma_start(out=lhsT_blk[64:128, t, :], in_=W1dT[:])

        pc = po_blk[:, 1, 0:ROWS]
        nc.tensor.matmul(pc, W1dT_sb, kTb_sb, start=True, stop=True)
        nc.scalar.activation(b1c, pc, AF.Identity, bias=b1v_sb, scale=-1.0)

        def emit_lhsT4(r0):
            """Build lhsT top halves for rows r0..r0+3 (r0 % 4 == 0)."""
            n = min(4, ROWS - r0)
            if n <= 0:
                return
            wb = W1pT_sb[:, None, :].broadcast_to([64, n, P])
            kb = kTb_sb[:, r0:r0 + n, None].broadcast_to([64, n, P])
            nc.vector.tensor_tensor(lhsT_blk[0:64, 0:n, :], wb, kb, ALU.mult)

        def emit_mm1(ii):
            lt = lhsT_blk[:, ii % 4, :]
            base = (ii % 2) * 768
            if ii % 2 == 0:
                nc.tensor.matmul(p1_blk[:, base:base + 512], lt, qq[:, 0:512],
                                 start=True, stop=True)
                nc.tensor.matmul(p1_blk[:, base + 512:base + 768], lt,
                                 qq[:, 512:768], start=True, stop=True)
            else:
                nc.tensor.matmul(p1_blk[:, base:base + 256], lt, qq[:, 0:256],
                                 start=True, stop=True)
                nc.tensor.matmul(p1_blk[:, base + 256:base + 768], lt,
                                 qq[:, 256:768], start=True, stop=True)

        g_t = [None, None, None]
        g2_t = [None, None, None]
        oct8_t = [None]

        def emit_mm2(row):
            """po + E2 matmuls for `row`, deferred one iteration so the PE
            queue never waits on same-row ACT/DVE output."""
            g = g_t[row % 3]
            g2 = g2_t[row % 3]
            po = po_blk[:, row % 5, 0:396]
            for c in range(6):
                nc.tensor.matmul(po[:, c * 66:c * 66 + 65],
                                 g[:, c * 128:(c + 1) * 128], W2ze_sb,
                                 start=True, stop=True)
            for c in range(6):
                nc.tensor.matmul(po[:, c * 66 + 65:c * 66 + 66],
                                 g2[:, c * 128:(c + 1) * 128],
                                 W2ze_sb[:, 64:65], start=True, stop=True)

        def emit_stats(lo):
            """Stats + finalize for rows lo..lo+3 (E2 of all 4 emitted).
            po banks (lo..lo+3) % 5 may wrap."""
            b0 = lo % 5
            stage = statsp.tile([P, 4, 6, 2], F32, tag="stage", name="stage")
            pv = po_blk[:, :, 0:396].rearrange(
                "p r (c w) -> p r c w", w=66)[:, :, :, 64:66]
            if b0 + 4 <= 5:
                nc.vector.tensor_copy(stage, pv[:, b0:b0 + 4])
            else:
                n1 = 5 - b0
                nc.vector.tensor_copy(stage[:, 0:n1], pv[:, b0:5])
                nc.vector.tensor_copy(stage[:, n1:4], pv[:, 0:4 - n1])
            mu2 = statsp.tile([P, 24], F32, tag="mu2", name="mu2")
            smu = stage[:, :, :, 0].rearrange("p r c -> p (r c)")
            se2 = stage[:, :, :, 1].rearrange("p r c -> p (r c)")
            nc.vector.scalar_tensor_tensor(mu2, smu, 1.0, smu, ALU.mult,
                                           ALU.mult)
            veps = statsp.tile([P, 24], F32, tag="veps", name="veps")
            nc.vector.scalar_tensor_tensor(veps, se2, EPS, mu2,
                                           ALU.add, ALU.subtract)
            # r = rsqrt(veps): bit-trick seed + 1 Newton step
            t1 = statsp.tile([P, 24], I32, tag="t1", name="t1")
            nc.vector.tensor_scalar(t1, veps[:].bitcast(I32), 1, None,
                                    ALU.logical_shift_right)
            r0 = statsp.tile([P, 24], F32, tag="r0", name="r0")
            nc.vector.tensor_scalar(r0[:].bitcast(I32), t1, -1, MAGIC,
                                    ALU.mult, ALU.add)
            qn = statsp.tile([P, 24], F32, tag="qn", name="qn")
            nc.vector.scalar_tensor_tensor(qn, r0, -0.5, r0, ALU.mult,
                                           ALU.mult)
            un = statsp.tile([P, 24], F32, tag="un", name="un")
            nc.vector.tensor_tensor(un, veps, qn, ALU.mult)
            r_sb = statsp.tile([P, 24], F32, tag="r", name="r")
            nc.vector.scalar_tensor_tensor(r_sb, un, 1.5, r0, ALU.add,
                                           ALU.mult)

            # queue per-row finalize work (osb spread one per iteration)
            if lo % 8 == 0:
                oct8_t[0] = outp.tile([P, 8, 6, 64], BF16, tag="oct8",
                                      name="oct8")
            for k in range(4):
                osb_q.append((lo + k, r_sb, k, oct8_t[0],
                              (lo % 8 != 0) * 4 + k, lo % 8 != 0 and k == 3,
                              lo))

        def emit_osb_one():
            if not osb_q:
                return
            row, r_sb, k, oct8, slot, last_of_8, lo = osb_q.pop(0)
            po_r = po_blk[:, row % 5, 0:396].rearrange("p (c w) -> p c w",
                                                       w=66)
            rb = r_sb[:, k * 6:k * 6 + 6, None].broadcast_to([P, 6, 64])
            nc.vector.tensor_mul(oct8[:, slot], po_r[:, :, 0:64], rb)
            if last_of_8:
                # o2 = oct8 + c for rows lo-4..lo+3; one DMA for 8 rows
                o2 = outp.tile([P, 8, 384], BF16, tag="o2", name="o2")
                cb = cfull_sb[:, None, :, :].broadcast_to([P, 8, 6, 64])
                nc.vector.tensor_tensor(
                    o2[:].rearrange("p r (c w) -> p r c w", w=64),
                    oct8[:], cb, ALU.add)
                nc.sync.dma_start(
                    out=out[lo - 4:lo + 4].rearrange("r (c p) n -> p r c n",
                                                     p=P),
                    in_=o2[:].rearrange("p r (c n) -> p r c n", n=NB))

        # ---- main loop ----
        osb_q = []
        emit_lhsT4(0)
        emit_mm1(0)
        for ii in range(ROWS):
            if ii % 4 == 3:
                emit_lhsT4(ii + 1)
            if ii + 1 < ROWS:
                emit_mm1(ii + 1)
            base = (ii % 2) * 768
            g = work.tile([P, L], BF16, tag="g", name="g")
            g_t[ii % 3] = g
            nc.scalar.activation(g, p1_blk[:, base:base + 768], AF.Gelu,
                                 bias=b1c[:, ii:ii + 1])
            g2 = work.tile([P, L], BF16, tag="g2", name="g2")
            g2_t[ii % 3] = g2
            if ii % 2 == 1:
                nc.gpsimd.tensor_tensor(g2, g, g, ALU.mult)
            else:
                nc.vector.tensor_mul(g2, g, g)

            if ii > 0:
                emit_mm2(ii - 1)
            if ii % 4 == 1 and ii >= 5:
                emit_stats(ii - 5)
            emit_osb_one()

        emit_mm2(ROWS - 1)
        emit_stats(ROWS - 4)
        while osb_q:
            emit_osb_one()


def host_prep(x, W_down, b_down, W1, b1, ln_g, ln_b, W2, b2):
    f32 = np.float32
    bf16 = ml_dtypes.bfloat16
    def swz(a):  # [1024, M] -> [128, 8, M] with row c*128+p -> [p, c]
        return np.ascontiguousarray(
            np.asarray(a, dtype=np.float32).reshape(8, P, -1)
            .transpose(1, 0, 2).astype(bf16))

    xTfull = np.ascontiguousarray(x[0].T.astype(f32))  # [D, L]
    common = {
        "xT": swz(xTfull),
        "WdTq": swz(W_down[:64, :].T),
        "WdTk": swz(W_down[64:, :].T),
        "bdq": np.ascontiguousarray(b_down[:64].astype(f32).reshape(64, 1)),
        "bdk": np.ascontiguousarray(b_down[64:].astype(f32).reshape(64, 1)),
        "W1pT": np.ascontiguousarray(W1[:, :64].T.astype(bf16)),
        "W1dT": np.ascontiguousarray(W1[:, 64:].T.astype(bf16)),
        "b1v": np.ascontiguousarray(b1.astype(f32).reshape(P, 1)),
    }
    W2g = W2.astype(np.float64) * ln_g.astype(np.float64)[None, :]
    W2z = W2g - W2g.mean(axis=1, keepdims=True)
    W2ze = np.concatenate([W2z.T, np.full((P, 1), 1.0 / 128.0)], axis=1)
    common["W2ze"] = np.ascontiguousarray(W2ze.astype(bf16))
    cvec = W2.astype(np.float64) @ ln_b.astype(np.float64) + b2.astype(np.float64)
    common["cfull"] = np.ascontiguousarray(
        np.tile(cvec[None, :], (P, 6)).astype(bf16))
    return common, xTfull


def kernel(x, W_down, b_down, W1, b1, ln_g, ln_b, W2, b2):
    x = np.asarray(x)
    common, xTfull = host_prep(
        x, np.asarray(W_down), np.asarray(b_down), np.asarray(W1),
        np.asarray(b1), np.asarray(ln_g), np.asarray(ln_b), np.asarray(W2),
        np.asarray(b2))

    nc = bacc.Bacc("TRN2")
    _build(nc)
    nc.finalize()

    in_maps = []
    for core in range(NCORES):
        m = dict(common)
        i0 = core * ROWS
        m["xTr"] = np.ascontiguousarray(
            xTfull[:, i0:i0 + ROWS].reshape(8, P, ROWS).transpose(1, 0, 2)
            .astype(ml_dtypes.bfloat16))
        in_maps.append(m)

    trace = os.environ.get("KERNEL_TRACE", "0") == "1"
    res = run_bass_kernel_spmd(nc, in_maps, core_ids=list(range(NCORES)),
                               trace=trace)
    global LAST_RES
    LAST_RES = res
    if trace and res.exec_time_ns is not None:
        print(f"HW exec time: {res.exec_time_ns} ns")
    outs = [res.results[c]["out"] for c in range(NCORES)]
    full = np.concatenate(outs, axis=0)  # [768, 768, 64]
    return full[None].astype(np.float32)


# revision 27
# speedup vs baseline: 1.0436x; 1.0436x over previous
"""Trainium2 Bass kernel for nn_PairwisePredictionHead (v3).

Math (reference):
  xd = x @ W_down.T + b_down             # [L, 128]
  q, k = xd[:, :64], xd[:, 64:]
  h[i,j,:] = W1p @ (q_j*k_i) + W1d @ (q_j - k_i) + b1    # [L, L, 128]
  g = gelu_exact(h)
  out = W2 @ LN(g) + b2                   # [L, L, 64]

Sharding: row-shard i across 8 cores (96 rows each); cores independent.

v3 changes vs baseline (204us):
 - E[g^2] via gram trick: per j-chunk, a second matmul gram_c = g_c.T@g_c
   (g_c already stationary) puts sum_h g^2 on the diagonal; read via a
   custom diagonal access pattern (dim0 stride = pitch+1). Removes the
   g2=g*g elementwise pass (DVE/ACT) and the 6 g2 weight loads.
 - rsqrt via fast-inverse-sqrt bit trick + 1 Newton step on DVE (5 small
   ops per 2 rows) instead of gpsimd pow (2.5us each, was the critical
   path). sqrt(128) scale folded into W2z host-side so the diag sum
   needs no /128.
 - PSUM: p1 double-buffer packed into 3 banks ([P,1536]; even rows split
   512/256, odd rows 256/512 to keep j-order contiguous), gram 2-row
   buffer 3 banks, po 2-row rotation 2 banks.
 - o2 (+c) and output DMA batched per 2 rows.
 - gpsimd micro-benchmarks in prep (overlapped with const DMAs) to
   calibrate real gpsimd op costs for future rebalancing.
"""

import os
from contextlib import ExitStack

import numpy as np
import ml_dtypes

import concourse.bass as bass
import concourse.mybir as mybir
import concourse.tile as tile
from concourse import bacc
from concourse.bass_utils import run_bass_kernel_spmd

F32 = mybir.dt.float32
I32 = mybir.dt.int32
BF16 = mybir.dt.bfloat16
ALU = mybir.AluOpType
AF = mybir.ActivationFunctionType

LAST_RES = None

B, L, D = 1, 768, 1024
DP, H, NB = 128, 128, 64
NCORES = 8
ROWS = L // NCORES  # 96 pair-grid rows per core
P = 128
EPS = 1e-5
MAGIC = 0x5F3759DF


def _build(nc):
    xT = nc.dram_tensor("xT", [P, 8, L], BF16, kind="ExternalInput")
    xTr = nc.dram_tensor("xTr", [P, 8, ROWS], BF16, kind="ExternalInput")
    WdTq = nc.dram_tensor("WdTq", [P, 8, 64], BF16, kind="ExternalInput")
    WdTk = nc.dram_tensor("WdTk", [P, 8, 64], BF16, kind="ExternalInput")
    bdq = nc.dram_tensor("bdq", [64, 1], F32, kind="ExternalInput")
    bdk = nc.dram_tensor("bdk", [64, 1], F32, kind="ExternalInput")
    W1pT = nc.dram_tensor("W1pT", [64, P], BF16, kind="ExternalInput")
    W1dT = nc.dram_tensor("W1dT", [64, P], BF16, kind="ExternalInput")
    b1v = nc.dram_tensor("b1v", [P, 1], F32, kind="ExternalInput")
    W2ze = nc.dram_tensor("W2ze", [P, 65], BF16, kind="ExternalInput")
    cfull = nc.dram_tensor("cfull", [P, 384], BF16, kind="ExternalInput")
    out = nc.dram_tensor("out", [ROWS, L, NB], BF16, kind="ExternalOutput")

    with tile.TileContext(nc) as tc, ExitStack() as ctx:
        const = ctx.enter_context(tc.tile_pool(name="const", bufs=1))
        work = ctx.enter_context(tc.tile_pool(name="work", bufs=3))
        outp = ctx.enter_context(tc.tile_pool(name="outp", bufs=2))
        statsp = ctx.enter_context(tc.tile_pool(name="statsp", bufs=2))
        ps1 = ctx.enter_context(tc.tile_pool(name="ps1", bufs=1, space="PSUM"))
        pso = ctx.enter_context(tc.tile_pool(name="pso", bufs=1, space="PSUM"))

        # ---- PSUM blocks (8 banks total) ----
        # p1 tiles are [P, 1024] so each occupies exactly 2 banks: no bank is
        # shared between the two buffers (bank-level serialization hazard).
        p1_t = [ps1.tile([P, 1024], F32, name=f"p1_{t}") for t in range(2)]
        po_blk = pso.tile([P, 4, 512], F32)      # 4 banks: po 4-row rotation

        # ---- gpsimd microbench (overlaps const DMAs; calibration only) ----
        mbs = const.tile([P, 390], F32)
        nc.gpsimd.memset(mbs, 1.0)
        mb1 = const.tile([64, P], BF16)
        nc.gpsimd.tensor_scalar_mul(mb1, mbs[0:64, 0:P], 2.0)
        mb2 = const.tile([P, 384], BF16)
        nc.gpsimd.tensor_tensor(mb2, mbs[:, 0:384], mbs[:, 0:384], ALU.add)
        mb3 = const.tile([P, 390], BF16)
        nc.gpsimd.tensor_copy(mb3, mbs)

        # ---- constants into SBUF (host pre-swizzled, contiguous DMAs) ----
        xT_sb = const.tile([P, 8, L], BF16)
        for c in range(8):
            nc.sync.dma_start(out=xT_sb[:, c, :], in_=xT[:, c, :])
        xTr_sb = const.tile([P, 8, ROWS], BF16)
        nc.sync.dma_start(out=xTr_sb, in_=xTr[:])
        WdTq_sb = const.tile([P, 8, 64], BF16)
        nc.sync.dma_start(out=WdTq_sb, in_=WdTq[:])
        WdTk_sb = const.tile([P, 8, 64], BF16)
        nc.sync.dma_start(out=WdTk_sb, in_=WdTk[:])
        bdq_sb = const.tile([64, 1], F32)
        nc.sync.dma_start(out=bdq_sb, in_=bdq[:])
        bdk_sb = const.tile([64, 1], F32)
        nc.sync.dma_start(out=bdk_sb, in_=bdk[:])
        W1pT_sb = const.tile([64, P], BF16)
        nc.sync.dma_start(out=W1pT_sb, in_=W1pT[:])
        W1dT_sb = const.tile([64, P], BF16)
        nc.sync.dma_start(out=W1dT_sb, in_=W1dT[:])
        b1v_sb = const.tile([P, 1], F32)
        nc.sync.dma_start(out=b1v_sb, in_=b1v[:])
        W2ze_sb = const.tile([P, 65], BF16)
        nc.sync.dma_start(out=W2ze_sb, in_=W2ze[:])
        cfull_sb = const.tile([P, 6, 64], BF16)
        nc.sync.dma_start(out=cfull_sb, in_=cfull[:].rearrange("p (c w) -> p c w", w=64))

        # ---- prep: qq = [q.T; q.T] (bf16), kT, b1c = b1 - W1d@kT ----
        qq = const.tile([P, L], BF16)
        kT_sb = const.tile([64, ROWS], F32)
        kTb_sb = const.tile([64, ROWS], BF16)
        b1c = const.tile([P, ROWS], F32)

        pq = p1_t[0][0:64, 0:768]
        for c in range(8):
            for h0, h1 in ((0, 512), (512, 768)):
                nc.tensor.matmul(
                    pq[:, h0:h1], WdTq_sb[:, c, :], xT_sb[:, c, h0:h1],
                    start=(c == 0), stop=(c == 7),
                )
        nc.scalar.activation(qq[0:64, :], pq, AF.Identity, bias=bdq_sb)
        nc.sync.dma_start(out=qq[64:128, :], in_=qq[0:64, :])

        pk = po_blk[0:64, 0, 0:ROWS]
        for c in range(8):
            nc.tensor.matmul(pk, WdTk_sb[:, c, :], xTr_sb[:, c, :],
                             start=(c == 0), stop=(c == 7))
        nc.scalar.activation(kT_sb, pk, AF.Identity, bias=bdk_sb)
        nc.vector.tensor_copy(kTb_sb, kT_sb)

        # persistent W1 stationary block: 4 rotating stationaries, bottom
        # halves static = W1d.T; top halves built 4 rows at a time on DVE.
        lhsT_blk = const.tile([P, 4, P], BF16, tag="lhsT", name="lhsT")
        for t in range(4):
            nc.sync.dma_start(out=lhsT_blk[64:128, t, :], in_=W1dT[:])

        pc = po_blk[:, 1, 0:ROWS]
        nc.tensor.matmul(pc, W1dT_sb, kTb_sb, start=True, stop=True)
        nc.scalar.activation(b1c, pc, AF.Identity, bias=b1v_sb, scale=-1.0)

        def emit_lhsT4(r0):
            """Build lhsT top halves for rows r0..r0+3 (r0 % 4 == 0)."""
            n = min(4, ROWS - r0)
            if n <= 0:
                return
            wb = W1pT_sb[:, None, :].broadcast_to([64, n, P])
            kb = kTb_sb[:, r0:r0 + n, None].broadcast_to([64, n, P])
            nc.vector.tensor_tensor(lhsT_blk[0:64, 0:n, :], wb, kb, ALU.mult)

        def emit_mm1(ii):
            lt = lhsT_blk[:, ii % 4, :]
            p1 = p1_t[ii % 2]
            nc.tensor.matmul(p1[:, 0:512], lt, qq[:, 0:512],
                             start=True, stop=True)
            nc.tensor.matmul(p1[:, 512:768], lt, qq[:, 512:768],
                             start=True, stop=True)

        g_t = [None, None, None]
        g2_t = [None, None, None]
        oct8_t = [None]

        def emit_mm2(row):
            """po + E2 matmuls for `row`, deferred one iteration so the PE
            queue never waits on same-row ACT/DVE output."""
            g = g_t[row % 3]
            g2 = g2_t[row % 3]
            po = po_blk[:, row % 4, 0:396]
            for c in range(6):
                nc.tensor.matmul(po[:, c * 66:c * 66 + 65],
                                 g[:, c * 128:(c + 1) * 128], W2ze_sb,
                                 start=(c == 0), stop=False)
            for c in range(6):
                nc.tensor.matmul(po[:, c * 66 + 65:c * 66 + 66],
                                 g2[:, c * 128:(c + 1) * 128],
                                 W2ze_sb[:, 64:65], start=False, stop=(c == 5))

        def emit_stats(lo):
            """Stats + finalize for rows lo..lo+3 (E2 of all 4 emitted).
            po banks lo%4..lo%4+3 (lo % 4 == 0: aligned)."""
            b0 = lo % 4
            stage = statsp.tile([P, 4, 6, 2], F32, tag="stage", name="stage")
            pv = po_blk[:, :, 0:396].rearrange(
                "p r (c w) -> p r c w", w=66)[:, :, :, 64:66]
            nc.vector.tensor_copy(stage, pv[:, b0:b0 + 4])
            mu2 = statsp.tile([P, 24], F32, tag="mu2", name="mu2")
            smu = stage[:, :, :, 0].rearrange("p r c -> p (r c)")
            se2 = stage[:, :, :, 1].rearrange("p r c -> p (r c)")
            nc.vector.scalar_tensor_tensor(mu2, smu, 1.0, smu, ALU.mult,
                                           ALU.mult)
            veps = statsp.tile([P, 24], F32, tag="veps", name="veps")
            nc.vector.scalar_tensor_tensor(veps, se2, EPS, mu2,
                                           ALU.add, ALU.subtract)
            # r = rsqrt(veps): bit-trick seed + 1 Newton step
            t1 = statsp.tile([P, 24], I32, tag="t1", name="t1")
            nc.vector.tensor_scalar(t1, veps[:].bitcast(I32), 1, None,
                                    ALU.logical_shift_right)
            r0 = statsp.tile([P, 24], F32, tag="r0", name="r0")
            nc.vector.tensor_scalar(r0[:].bitcast(I32), t1, -1, MAGIC,
                                    ALU.mult, ALU.add)
            qn = statsp.tile([P, 24], F32, tag="qn", name="qn")
            nc.vector.scalar_tensor_tensor(qn, r0, -0.5, r0, ALU.mult,
                                           ALU.mult)
            un = statsp.tile([P, 24], F32, tag="un", name="un")
            nc.vector.tensor_tensor(un, veps, qn, ALU.mult)
            r_sb = statsp.tile([P, 24], F32, tag="r", name="r")
            nc.vector.scalar_tensor_tensor(r_sb, un, 1.5, r0, ALU.add,
                                           ALU.mult)

            # queue per-row finalize work (osb spread one per iteration)
            if lo % 8 == 0:
                oct8_t[0] = outp.tile([P, 8, 6, 64], BF16, tag="oct8",
                                      name="oct8")
            for k in range(4):
                osb_q.append((lo + k, r_sb, k, oct8_t[0],
                              (lo % 8 != 0) * 4 + k, lo % 8 != 0 and k == 3,
                              lo))

        def emit_osb_one():
            if not osb_q:
                return
            row, r_sb, k, oct8, slot, last_of_8, lo = osb_q.pop(0)
            po_r = po_blk[:, row % 4, 0:396].rearrange("p (c w) -> p c w",
                                                       w=66)
            rb = r_sb[:, k * 6:k * 6 + 6, None].broadcast_to([P, 6, 64])
            nc.vector.tensor_mul(oct8[:, slot], po_r[:, :, 0:64], rb)
            if last_of_8:
                # o2 = oct8 + c for rows lo-4..lo+3; one DMA for 8 rows
                o2 = outp.tile([P, 8, 384], BF16, tag="o2", name="o2")
                cb = cfull_sb[:, None, :, :].broadcast_to([P, 8, 6, 64])
                nc.vector.tensor_tensor(
                    o2[:].rearrange("p r (c w) -> p r c w", w=64),
                    oct8[:], cb, ALU.add)
                nc.sync.dma_start(
                    out=out[lo - 4:lo + 4].rearrange("r (c p) n -> p r c n",
                                                     p=P),
                    in_=o2[:].rearrange("p r (c n) -> p r c n", n=NB))

        # ---- main loop ----
        osb_q = []
        emit_lhsT4(0)
        emit_mm1(0)
        for ii in range(ROWS):
            if ii % 4 == 3:
                emit_lhsT4(ii + 1)
            if ii + 1 < ROWS:
                emit_mm1(ii + 1)
            g = work.tile([P, L], BF16, tag="g", name="g")
            g_t[ii % 3] = g
            nc.scalar.activation(g, p1_t[ii % 2][:, 0:768], AF.Gelu,
                                 bias=b1c[:, ii:ii + 1])
            g2 = work.tile([P, L], BF16, tag="g2", name="g2")
            g2_t[ii % 3] = g2
            if ii % 2 == 1:
                nc.gpsimd.tensor_tensor(g2, g, g, ALU.mult)
            else:
                nc.vector.tensor_mul(g2, g, g)

            if ii > 0:
                emit_mm2(ii - 1)
            if ii % 4 == 0 and ii >= 4:
                emit_stats(ii - 4)
            emit_osb_one()

        emit_mm2(ROWS - 1)
        emit_stats(ROWS - 4)
        while osb_q:
            emit_osb_one()


def host_prep(x, W_down, b_down, W1, b1, ln_g, ln_b, W2, b2):
    f32 = np.float32
    bf16 = ml_dtypes.bfloat16
    def swz(a):  # [1024, M] -> [128, 8, M] with row c*128+p -> [p, c]
        return np.ascontiguousarray(
            np.asarray(a, dtype=np.float32).reshape(8, P, -1)
            .transpose(1, 0, 2).astype(bf16))

    xTfull = np.ascontiguousarray(x[0].T.astype(f32))  # [D, L]
    common = {
        "xT": swz(xTfull),
        "WdTq": swz(W_down[:64, :].T),
        "WdTk": swz(W_down[64:, :].T),
        "bdq": np.ascontiguousarray(b_down[:64].astype(f32).reshape(64, 1)),
        "bdk": np.ascontiguousarray(b_down[64:].astype(f32).reshape(64, 1)),
        "W1pT": np.ascontiguousarray(W1[:, :64].T.astype(bf16)),
        "W1dT": np.ascontiguousarray(W1[:, 64:].T.astype(bf16)),
        "b1v": np.ascontiguousarray(b1.astype(f32).reshape(P, 1)),
    }
    W2g = W2.astype(np.float64) * ln_g.astype(np.float64)[None, :]
    W2z = W2g - W2g.mean(axis=1, keepdims=True)
    W2ze = np.concatenate([W2z.T, np.full((P, 1), 1.0 / 128.0)], axis=1)
    common["W2ze"] = np.ascontiguousarray(W2ze.astype(bf16))
    cvec = W2.astype(np.float64) @ ln_b.astype(np.float64) + b2.astype(np.float64)
    common["cfull"] = np.ascontiguousarray(
        np.tile(cvec[None, :], (P, 6)).astype(bf16))
    return common, xTfull


def kernel(x, W_down, b_down, W1, b1, ln_g, ln_b, W2, b2):
    x = np.asarray(x)
    common, xTfull = host_prep(
        x, np.asarray(W_down), np.asarray(b_down), np.asarray(W1),
        np.asarray(b1), np.asarray(ln_g), np.asarray(ln_b), np.asarray(W2),
        np.asarray(b2))

    nc = bacc.Bacc("TRN2")
    _build(nc)
    nc.finalize()

    in_maps = []
    for core in range(NCORES):
        m = dict(common)
        i0 = core * ROWS
        m["xTr"] = np.ascontiguousarray(
            xTfull[:, i0:i0 + ROWS].reshape(8, P, ROWS).transpose(1, 0, 2)
            .astype(ml_dtypes.bfloat16))
        in_maps.append(m)

    trace = os.environ.get("KERNEL_TRACE", "0") == "1"
    res = run_bass_kernel_spmd(nc, in_maps, core_ids=list(range(NCORES)),
                               trace=trace)
    global LAST_RES
    LAST_RES = res
    if trace and res.exec_time_ns is not None:
        print(f"HW exec time: {res.exec_time_ns} ns")
    outs = [res.results[c]["out"] for c in range(NCORES)]
    full = np.concatenate(outs, axis=0)  # [768, 768, 64]
    return full[None].astype(np.float32)
